# revision 7
# baseline (speedup 1.0000x reference)
"""Bass/Tile TRN2 kernel for BasicAttention.

att = softmax(tanh(hidden @ W_h.T + p_att_feats) @ W_alpha + mask) @ att_feats

Shapes: B=64, N=2048, H=1024, A=512. Data-parallel over batch across 8
NeuronCores (8 batches per core); weights replicated; no collectives.

Layout: region index n maps to (partition p, column c) as n = p*16 + c so
every p_att/att_feats DMA is a long contiguous per-partition read.

Per-core dataflow (memory-bound: ~100.7MB HBM reads/core, ~355 GB/s/core
practical cap with all 8 cores streaming — measured with a pure-DMA
microbenchmark):
  host: pass W_h.T / hidden.T / masks pre-rearranged so each setup load is
        one contiguous DMA; pre-broadcast bf16 W_alpha.
  setup (no DRAM round-trips, nothing blocking the stream queue):
    w_h = hidden @ W_h.T on PE, then per-batch partition-broadcast of
    w_h rows via one-hot PE matmuls (PSUM -> SBUF copies on ACT).
  per batch b (software-pipelined, p_att phase leads att_feats phase):
    p_att [128,16,512] in ONE 4MB DMA (32KB/partition contiguous):
      DVE add (w_h bcast) -> ACT tanh (bf16, 2 halves) -> DVE
      scalar_tensor_tensor vs W_alpha (accum) -> scores[128,16]
    scores: + mask, ACT exp (accum rowsum, f32r out), PE total-sum,
      DVE reciprocal
    att_feats [128,8,1024] f32r in TWO 4MB DMAs: PE matmuls (expt col
      stationary) accumulating att[1,1024] in PSUM -> DVE scale by
      1/sum -> out row.
  Stream DMAs ride the SP HWDGE queue; setup/output DMAs ride the ACT
  HWDGE queue so they never stall the stream.
"""

import numpy as np

B, N, H, A = 64, 2048, 1024, 512
NCORES = 8
BLOC = B // NCORES  # batches per core

P = 128
NT = N // P            # 16 n-columns per partition
HC = H // P            # 8 h-chunks

_NC_CACHE = {}


def _free_bcast(bass_mod, ap, repeat):
    """[P, F] AP -> [P, repeat, F] AP with 0-stride middle dim."""
    return bass_mod.AP(
        tensor=ap.tensor,
        offset=ap.offset,
        ap=[ap.ap[0], [0, repeat], *ap.ap[1:]],
    )


def _build_nc():
    import concourse.bass as bass
    import concourse.mybir as mybir
    import concourse.tile as tile
    from concourse import bacc

    dt = mybir.dt
    f32, f32r, bf16 = dt.float32, dt.float32r, dt.bfloat16
    AF = mybir.ActivationFunctionType
    OP = mybir.AluOpType

    nc = bacc.Bacc("TRN2", target_bir_lowering=False, debug=False,
                   num_devices=NCORES)

    af = nc.dram_tensor("att_feats", [BLOC, N, H], f32r, kind="ExternalInput").ap()
    pa = nc.dram_tensor("p_att_feats", [BLOC, N, A], f32, kind="ExternalInput").ap()
    # host-rearranged: masks_r[p, b, c] = att_masks[b, p*16+c]
    am = nc.dram_tensor("masks_r", [P, BLOC, NT], f32, kind="ExternalInput").ap()
    # host-rearranged: whT_r[p, hc, a] = W_h[a, hc*128+p]
    whT = nc.dram_tensor("whT_r", [P, HC, A], f32, kind="ExternalInput").ap()
    # host-rearranged: hidT_r[p, hc*8+b] = hidden[b, hc*128+p]
    hsT = nc.dram_tensor("hidT_r", [P, HC * BLOC], f32, kind="ExternalInput").ap()
    wab = nc.dram_tensor("W_alpha_b", [P, A], bf16, kind="ExternalInput").ap()
    # onehot[k, b*128+p] = 1 if k == b else 0 (for the w_h row broadcast)
    oh = nc.dram_tensor("onehot", [BLOC, BLOC * P], f32, kind="ExternalInput").ap()
    out = nc.dram_tensor("att_out", [BLOC, H], f32, kind="ExternalOutput").ap()

    with tile.TileContext(nc) as tc:
        with (
            tc.tile_pool(name="consts", bufs=1) as consts,
            tc.tile_pool(name="whtp", bufs=2) as whtp,
            tc.tile_pool(name="patt", bufs=2) as patt_pool,
            tc.tile_pool(name="alpha", bufs=2) as alpha_pool,
            tc.tile_pool(name="afp", bufs=2) as af_pool,
            tc.tile_pool(name="small", bufs=3) as small,
            tc.tile_pool(name="psmisc", bufs=2, space="PSUM") as psmisc,
            tc.tile_pool(name="psatt", bufs=6, space="PSUM") as psatt,
        ):
            # ---------------- setup (ACT HWDGE queue, PE, no DRAM trips) ----
            ones_col = consts.tile([P, 1], f32)
            nc.vector.memset(ones_col, 1.0)

            wa_bf = consts.tile([P, A], bf16)
            nc.scalar.dma_start(out=wa_bf, in_=wab[:, :])
            hidT = consts.tile([P, HC * BLOC], f32)
            nc.scalar.dma_start(out=hidT, in_=hsT[:, :])
            masks = consts.tile([P, BLOC, NT], f32)
            nc.scalar.dma_start(out=masks, in_=am[:, :, :])

            onehot = consts.tile([BLOC, BLOC * P], f32)
            nc.scalar.dma_start(out=onehot, in_=oh[:, :])

            # w_h = hidden @ W_h.T : [8, 512]
            wh_ps = psmisc.tile([BLOC, A], f32, tag="mm")
            for hc in range(HC):
                wt = whtp.tile([P, 1, A], f32, tag="whT", name=f"whT{hc}")
                nc.scalar.dma_start(out=wt, in_=whT[:, hc:hc + 1, :])
                nc.tensor.matmul(wh_ps, lhsT=hidT[:, hc * BLOC:(hc + 1) * BLOC],
                                 rhs=wt[:, 0, :],
                                 start=(hc == 0), stop=(hc == HC - 1))
            whall_sb = consts.tile([BLOC, A], f32)
            nc.scalar.activation(whall_sb, wh_ps, AF.Copy)

            # per-batch w_h row broadcast to [128, 512] via one-hot matmuls
            whb = []
            for b in range(BLOC):
                bps = psmisc.tile([P, A], f32, tag="mm", name=f"whbps{b}")
                nc.tensor.matmul(bps, lhsT=onehot[:, b * P:(b + 1) * P],
                                 rhs=whall_sb, start=True, stop=True)
                t = consts.tile([P, A], f32, name=f"whb{b}", tag=f"whb{b}")
                nc.scalar.activation(t, bps, AF.Copy)
                whb.append(t)

            # ---------------- main loop (software-pipelined) ----------------
            # n = p*NT + c everywhere below.
            pa_r = [pa[b, :, :].rearrange("(p c) a -> p c a", c=NT)
                    for b in range(BLOC)]
            af_r = [af[b, :, :].rearrange("(p c) h -> p c h", c=NT)
                    for b in range(BLOC)]

            def patt_phase(b):
                pt = patt_pool.tile([P, NT, A], f32, tag="patt",
                                    name=f"patt{b}")
                nc.sync.dma_start(out=pt, in_=pa_r[b][:, :, :])
                whb_b = _free_bcast(bass, whb[b][:, :], NT)
                nc.vector.tensor_tensor(out=pt, in0=pt, in1=whb_b, op=OP.add)

                scores = small.tile([P, NT], f32, tag="scores",
                                    name=f"scores{b}")
                HALF = NT // 2
                for st in range(2):
                    ab = alpha_pool.tile([P, HALF, A], bf16, tag="alpha",
                                         name=f"alpha{b}_{st}")
                    nc.scalar.activation(
                        ab, pt[:, st * HALF:(st + 1) * HALF, :], AF.Tanh)
                    for c in range(HALF):
                        col = st * HALF + c
                        # out = (ab * 1) * wa ; accum_out = row-sum -> scores
                        nc.vector.scalar_tensor_tensor(
                            out=ab[:, c, :], in0=ab[:, c, :], scalar=1.0,
                            in1=wa_bf, op0=OP.mult, op1=OP.mult,
                            accum_out=scores[:, col:col + 1],
                        )

                nc.vector.tensor_tensor(out=scores, in0=scores,
                                        in1=masks[:, b, :], op=OP.add)

                expt = small.tile([P, NT], f32r, tag="expt", name=f"expt{b}")
                rowsum = small.tile([P, 1], f32, tag="rowsum", name=f"rowsum{b}")
                nc.scalar.activation(expt, scores, AF.Exp, accum_out=rowsum)

                sum_ps = psmisc.tile([1, 1], f32, tag="mm", name=f"sum_ps{b}")
                nc.tensor.matmul(sum_ps, lhsT=rowsum, rhs=ones_col,
                                 start=True, stop=True)
                inv = small.tile([1, 1], f32, tag="inv", name=f"inv{b}")
                nc.vector.reciprocal(inv, sum_ps)
                return expt, inv

            AF_SUP = 8  # columns per att_feats supertile (2 DMAs per batch)

            def af_phase(b, expt, inv):
                att_lo = psatt.tile([1, A], f32, tag="att", name=f"attlo{b}")
                att_hi = psatt.tile([1, A], f32, tag="att", name=f"atthi{b}")
                for st2 in range(NT // AF_SUP):
                    aft = af_pool.tile([P, AF_SUP, H], f32r, tag="af",
                                       name=f"af{b}_{st2}")
                    nc.sync.dma_start(
                        out=aft,
                        in_=af_r[b][:, st2 * AF_SUP:(st2 + 1) * AF_SUP, :],
                    )
                    for c in range(AF_SUP):
                        t = st2 * AF_SUP + c
                        lhs = expt[:, t:t + 1]
                        nc.tensor.matmul(att_lo, lhsT=lhs,
                                         rhs=aft[:, c, 0:A],
                                         start=(t == 0), stop=(t == NT - 1))
                        nc.tensor.matmul(att_hi, lhsT=lhs,
                                         rhs=aft[:, c, A:H],
                                         start=(t == 0), stop=(t == NT - 1))

                att_row = small.tile([1, H], f32, tag="attrow",
                                     name=f"attrow{b}")
                nc.vector.tensor_scalar_mul(att_row[:, 0:A], att_lo, inv)
                nc.vector.tensor_scalar_mul(att_row[:, A:H], att_hi, inv)
                nc.scalar.dma_start(out=out[b:b + 1, :], in_=att_row)

            state = {}
            for b in range(BLOC):
                state[b] = patt_phase(b)
                if b >= 1:
                    af_phase(b - 1, *state.pop(b - 1))
            af_phase(BLOC - 1, *state.pop(BLOC - 1))

    nc.compile()
    return nc


def _get_nc():
    if "nc" not in _NC_CACHE:
        _NC_CACHE["nc"] = _build_nc()
    return _NC_CACHE["nc"]


def kernel(hidden_states, att_feats, p_att_feats, att_masks, W_h, W_alpha):
    import ml_dtypes
    from concourse.bass_utils import run_bass_kernel_spmd

    nc = _get_nc()
    hidden_states = np.ascontiguousarray(hidden_states, dtype=np.float32)
    att_feats = np.ascontiguousarray(att_feats, dtype=np.float32)
    p_att_feats = np.ascontiguousarray(p_att_feats, dtype=np.float32)
    att_masks = np.ascontiguousarray(att_masks, dtype=np.float32)
    W_h = np.ascontiguousarray(W_h, dtype=np.float32)
    W_alpha = np.asarray(W_alpha, dtype=np.float32).reshape(1, A)

    # whT_r[p, hc, a] = W_h[a, hc*128+p]
    whT_r = np.ascontiguousarray(
        W_h.T.reshape(HC, P, A).transpose(1, 0, 2))
    wab = np.ascontiguousarray(
        np.broadcast_to(W_alpha, (P, A))).astype(ml_dtypes.bfloat16)
    onehot_host = np.zeros((BLOC, BLOC * P), dtype=np.float32)
    for b in range(BLOC):
        onehot_host[b, b * P:(b + 1) * P] = 1.0

    in_maps = []
    for i in range(NCORES):
        s = slice(i * BLOC, (i + 1) * BLOC)
        hs = hidden_states[s]  # [BLOC, H]
        # hidT_r[p, hc*8+b] = hidden[b, hc*128+p]
        hidT_r = np.ascontiguousarray(
            hs.T.reshape(HC, P, BLOC).transpose(1, 0, 2).reshape(P, HC * BLOC))
        masks_r = np.ascontiguousarray(
            att_masks[s].reshape(BLOC, P, NT).transpose(1, 0, 2))
        in_maps.append({
            "att_feats": att_feats[s],
            "p_att_feats": p_att_feats[s],
            "masks_r": masks_r,
            "whT_r": whT_r,
            "hidT_r": hidT_r,
            "W_alpha_b": wab,
            "onehot": onehot_host,
        })

    global _LAST_IN_MAPS
    _LAST_IN_MAPS = in_maps
    res = run_bass_kernel_spmd(nc, in_maps, core_ids=list(range(NCORES)))
    return np.concatenate(
        [res.results[i]["att_out"] for i in range(NCORES)], axis=0
    ).astype(np.float32)


_LAST_IN_MAPS = None


# revision 11
# speedup vs baseline: 1.0683x; 1.0683x over previous
"""Bass/Tile TRN2 kernel for BasicAttention.

att = softmax(tanh(hidden @ W_h.T + p_att_feats) @ W_alpha + mask) @ att_feats

Shapes: B=64, N=2048, H=1024, A=512. Data-parallel over batch across 8
NeuronCores (8 batches per core); weights replicated; no collectives.

Layout: region index n maps to (partition p, column c) as n = p*16 + c so
every p_att/att_feats DMA is a long contiguous per-partition read.

Per-core dataflow (memory-bound: ~100.7MB HBM reads/core, ~355 GB/s/core
practical cap with all 8 cores streaming — measured with a pure-DMA
microbenchmark):
  host: pass W_h.T / hidden.T / masks pre-rearranged so each setup load is
        one contiguous DMA; pre-broadcast bf16 W_alpha.
  setup (no DRAM round-trips, nothing blocking the stream queue):
    w_h = hidden @ W_h.T on PE, then per-batch partition-broadcast of
    w_h rows via one-hot PE matmuls (PSUM -> SBUF copies on ACT).
  per batch b (software-pipelined, p_att phase leads att_feats phase):
    p_att [128,16,512] in ONE 4MB DMA (32KB/partition contiguous):
      DVE add (w_h bcast) -> ACT tanh (bf16, 2 halves) -> DVE
      scalar_tensor_tensor vs W_alpha (accum) -> scores[128,16]
    scores: + mask, ACT exp (accum rowsum, f32r out), PE total-sum,
      DVE reciprocal
    att_feats [128,8,1024] f32r in TWO 4MB DMAs: PE matmuls (expt col
      stationary) accumulating att[1,1024] in PSUM -> DVE scale by
      1/sum -> out row.
  Stream DMAs ride the SP HWDGE queue; setup/output DMAs ride the ACT
  HWDGE queue so they never stall the stream.
"""

import numpy as np

B, N, H, A = 64, 2048, 1024, 512
NCORES = 8
BLOC = B // NCORES  # batches per core

P = 128
NT = N // P            # 16 n-columns per partition
HC = H // P            # 8 h-chunks

_NC_CACHE = {}


def _free_bcast(bass_mod, ap, repeat):
    """[P, F] AP -> [P, repeat, F] AP with 0-stride middle dim."""
    return bass_mod.AP(
        tensor=ap.tensor,
        offset=ap.offset,
        ap=[ap.ap[0], [0, repeat], *ap.ap[1:]],
    )


def _build_nc():
    import concourse.bass as bass
    import concourse.mybir as mybir
    import concourse.tile as tile
    from concourse import bacc

    dt = mybir.dt
    f32, f32r, bf16 = dt.float32, dt.float32r, dt.bfloat16
    AF = mybir.ActivationFunctionType
    OP = mybir.AluOpType

    nc = bacc.Bacc("TRN2", target_bir_lowering=False, debug=False,
                   num_devices=NCORES)

    af = nc.dram_tensor("att_feats", [BLOC, N, H], f32r, kind="ExternalInput").ap()
    pa = nc.dram_tensor("p_att_feats", [BLOC, N, A], f32, kind="ExternalInput").ap()
    # host-rearranged: masks_r[p, b, c] = att_masks[b, p*16+c]
    am = nc.dram_tensor("masks_r", [P, BLOC, NT], f32, kind="ExternalInput").ap()
    # host-rearranged: whT_r[p, hc, a] = W_h[a, hc*128+p]
    whT = nc.dram_tensor("whT_r", [P, HC, A], f32, kind="ExternalInput").ap()
    # host-rearranged: hidT_r[p, hc*8+b] = hidden[b, hc*128+p]
    hsT = nc.dram_tensor("hidT_r", [P, HC * BLOC], f32, kind="ExternalInput").ap()
    wab = nc.dram_tensor("W_alpha_b", [P, A], bf16, kind="ExternalInput").ap()
    # onehot[k, b*128+p] = 1 if k == b else 0 (for the w_h row broadcast)
    oh = nc.dram_tensor("onehot", [BLOC, BLOC * P], f32, kind="ExternalInput").ap()
    out = nc.dram_tensor("att_out", [BLOC, H], f32, kind="ExternalOutput").ap()

    with tile.TileContext(nc) as tc:
        with (
            tc.tile_pool(name="consts", bufs=1) as consts,
            tc.tile_pool(name="whtp", bufs=2) as whtp,
            tc.tile_pool(name="patt", bufs=2) as patt_pool,
            tc.tile_pool(name="alpha", bufs=2) as alpha_pool,
            tc.tile_pool(name="afp", bufs=2) as af_pool,
            tc.tile_pool(name="small", bufs=3) as small,
            tc.tile_pool(name="arow", bufs=2) as arow,
            tc.tile_pool(name="psmisc", bufs=2, space="PSUM") as psmisc,
            tc.tile_pool(name="psatt", bufs=6, space="PSUM") as psatt,
        ):
            # ---------------- setup (ACT HWDGE queue, PE, no DRAM trips) ----
            ones_col = consts.tile([P, 1], f32)
            nc.vector.memset(ones_col, 1.0)

            wa_bf = consts.tile([P, A], bf16)
            nc.scalar.dma_start(out=wa_bf, in_=wab[:, :])
            hidT = consts.tile([P, HC * BLOC], f32)
            nc.scalar.dma_start(out=hidT, in_=hsT[:, :])
            masks = consts.tile([P, BLOC, NT], f32)
            nc.scalar.dma_start(out=masks, in_=am[:, :, :])

            onehot = consts.tile([BLOC, BLOC * P], f32)
            nc.scalar.dma_start(out=onehot, in_=oh[:, :])

            # w_h = hidden @ W_h.T : [8, 512]
            wh_ps = psmisc.tile([BLOC, A], f32, tag="mm")
            for hc in range(HC):
                wt = whtp.tile([P, 1, A], f32, tag="whT", name=f"whT{hc}")
                nc.scalar.dma_start(out=wt, in_=whT[:, hc:hc + 1, :])
                nc.tensor.matmul(wh_ps, lhsT=hidT[:, hc * BLOC:(hc + 1) * BLOC],
                                 rhs=wt[:, 0, :],
                                 start=(hc == 0), stop=(hc == HC - 1))
            whall_sb = consts.tile([BLOC, A], f32)
            nc.scalar.activation(whall_sb, wh_ps, AF.Copy)

            # per-batch w_h row broadcast to [128, 512] via one-hot matmuls
            whb = []
            for b in range(BLOC):
                bps = psmisc.tile([P, A], f32, tag="mm", name=f"whbps{b}")
                nc.tensor.matmul(bps, lhsT=onehot[:, b * P:(b + 1) * P],
                                 rhs=whall_sb, start=True, stop=True)
                t = consts.tile([P, A], f32, name=f"whb{b}", tag=f"whb{b}")
                nc.scalar.activation(t, bps, AF.Copy)
                whb.append(t)

            # ---------------- main loop (software-pipelined) ----------------
            # n = p*NT + c everywhere below.
            pa_r = [pa[b, :, :].rearrange("(p c) a -> p c a", c=NT)
                    for b in range(BLOC)]
            af_r = [af[b, :, :].rearrange("(p c) h -> p c h", c=NT)
                    for b in range(BLOC)]

            def patt_front(b):
                """DMA + add + tanh + stt + mask -> scores tile."""
                pt = patt_pool.tile([P, NT, A], f32, tag="patt",
                                    name=f"patt{b}")
                nc.sync.dma_start(out=pt, in_=pa_r[b][:, :, :])

                scores = small.tile([P, NT], f32, tag="scores",
                                    name=f"scores{b}")
                HALF = NT // 2
                whb_b = _free_bcast(bass, whb[b][:, :], HALF)
                for st in range(2):
                    sl = pt[:, st * HALF:(st + 1) * HALF, :]
                    nc.vector.tensor_tensor(out=sl, in0=sl, in1=whb_b,
                                            op=OP.add)
                    ab = alpha_pool.tile([P, HALF, A], bf16, tag="alpha",
                                         name=f"alpha{b}_{st}")
                    nc.scalar.activation(ab, sl, AF.Tanh)
                    for c in range(HALF):
                        col = st * HALF + c
                        # out = (ab * 1) * wa ; accum_out = row-sum -> scores
                        nc.vector.scalar_tensor_tensor(
                            out=ab[:, c, :], in0=ab[:, c, :], scalar=1.0,
                            in1=wa_bf, op0=OP.mult, op1=OP.mult,
                            accum_out=scores[:, col:col + 1],
                        )

                nc.vector.tensor_tensor(out=scores, in0=scores,
                                        in1=masks[:, b, :], op=OP.add)
                return scores

            def patt_fin(b, scores):
                """exp + total-sum + reciprocal. Issued AFTER af_phase(b-1)
                so the sum matmul never sits ahead of batch b-1's att
                matmuls in the PE FIFO."""
                expt = small.tile([P, NT], f32r, tag="expt", name=f"expt{b}")
                rowsum = small.tile([P, 1], f32, tag="rowsum", name=f"rowsum{b}")
                nc.scalar.activation(expt, scores, AF.Exp, accum_out=rowsum)

                sum_ps = psmisc.tile([1, 1], f32, tag="mm", name=f"sum_ps{b}")
                nc.tensor.matmul(sum_ps, lhsT=rowsum, rhs=ones_col,
                                 start=True, stop=True)
                inv = small.tile([1, 1], f32, tag="inv", name=f"inv{b}")
                nc.vector.reciprocal(inv, sum_ps)
                return expt, inv

            AF_SUP = 8  # columns per att_feats supertile (2 DMAs per batch)

            def af_phase(b, expt, inv):
                att_lo = psatt.tile([1, A], f32, tag="att", name=f"attlo{b}")
                att_hi = psatt.tile([1, A], f32, tag="att", name=f"atthi{b}")
                for st2 in range(NT // AF_SUP):
                    aft = af_pool.tile([P, AF_SUP, H], f32r, tag="af",
                                       name=f"af{b}_{st2}")
                    nc.sync.dma_start(
                        out=aft,
                        in_=af_r[b][:, st2 * AF_SUP:(st2 + 1) * AF_SUP, :],
                    )
                    for c in range(AF_SUP):
                        t = st2 * AF_SUP + c
                        lhs = expt[:, t:t + 1]
                        nc.tensor.matmul(att_lo, lhsT=lhs,
                                         rhs=aft[:, c, 0:A],
                                         start=(t == 0), stop=(t == NT - 1))
                        nc.tensor.matmul(att_hi, lhsT=lhs,
                                         rhs=aft[:, c, A:H],
                                         start=(t == 0), stop=(t == NT - 1))

                att_row = arow.tile([1, H], f32, tag="attrow",
                                     name=f"attrow{b}")
                nc.vector.tensor_scalar_mul(att_row[:, 0:A], att_lo, inv)
                nc.vector.tensor_scalar_mul(att_row[:, A:H], att_hi, inv)
                # SWDGE queue (idle GpSimd): keeps the tiny output store off
                # both HWDGE rings so it can't head-of-line-block anything.
                nc.gpsimd.dma_start(out=out[b:b + 1, :], in_=att_row)

            fin = {}
            for b in range(BLOC):
                scores_b = patt_front(b)
                if b >= 1:
                    af_phase(b - 1, *fin.pop(b - 1))
                fin[b] = patt_fin(b, scores_b)
            af_phase(BLOC - 1, *fin.pop(BLOC - 1))

    nc.compile()
    return nc


def _get_nc():
    if "nc" not in _NC_CACHE:
        _NC_CACHE["nc"] = _build_nc()
    return _NC_CACHE["nc"]


def kernel(hidden_states, att_feats, p_att_feats, att_masks, W_h, W_alpha):
    import ml_dtypes
    from concourse.bass_utils import run_bass_kernel_spmd

    nc = _get_nc()
    hidden_states = np.ascontiguousarray(hidden_states, dtype=np.float32)
    att_feats = np.ascontiguousarray(att_feats, dtype=np.float32)
    p_att_feats = np.ascontiguousarray(p_att_feats, dtype=np.float32)
    att_masks = np.ascontiguousarray(att_masks, dtype=np.float32)
    W_h = np.ascontiguousarray(W_h, dtype=np.float32)
    W_alpha = np.asarray(W_alpha, dtype=np.float32).reshape(1, A)

    # whT_r[p, hc, a] = W_h[a, hc*128+p]
    whT_r = np.ascontiguousarray(
        W_h.T.reshape(HC, P, A).transpose(1, 0, 2))
    wab = np.ascontiguousarray(
        np.broadcast_to(W_alpha, (P, A))).astype(ml_dtypes.bfloat16)
    onehot_host = np.zeros((BLOC, BLOC * P), dtype=np.float32)
    for b in range(BLOC):
        onehot_host[b, b * P:(b + 1) * P] = 1.0

    in_maps = []
    for i in range(NCORES):
        s = slice(i * BLOC, (i + 1) * BLOC)
        hs = hidden_states[s]  # [BLOC, H]
        # hidT_r[p, hc*8+b] = hidden[b, hc*128+p]
        hidT_r = np.ascontiguousarray(
            hs.T.reshape(HC, P, BLOC).transpose(1, 0, 2).reshape(P, HC * BLOC))
        masks_r = np.ascontiguousarray(
            att_masks[s].reshape(BLOC, P, NT).transpose(1, 0, 2))
        in_maps.append({
            "att_feats": att_feats[s],
            "p_att_feats": p_att_feats[s],
            "masks_r": masks_r,
            "whT_r": whT_r,
            "hidT_r": hidT_r,
            "W_alpha_b": wab,
            "onehot": onehot_host,
        })

    global _LAST_IN_MAPS
    _LAST_IN_MAPS = in_maps
    res = run_bass_kernel_spmd(nc, in_maps, core_ids=list(range(NCORES)))
    return np.concatenate(
        [res.results[i]["att_out"] for i in range(NCORES)], axis=0
    ).astype(np.float32)


_LAST_IN_MAPS = None


# revision 12
# speedup vs baseline: 1.2626x; 1.1819x over previous
"""Bass/Tile TRN2 kernel for BasicAttention.

att = softmax(tanh(hidden @ W_h.T + p_att_feats) @ W_alpha + mask) @ att_feats

Shapes: B=64, N=2048, H=1024, A=512. Data-parallel over batch across 8
NeuronCores (8 batches per core); weights replicated; no collectives.

Layout: region index n maps to (partition p, column c) as n = p*16 + c so
every p_att/att_feats DMA is a long contiguous per-partition read.

Per-core dataflow (memory-bound: ~100.7MB HBM reads/core; ~369 GB/s/core
steady-state streaming measured with a pure-DMA microbenchmark; all input
bytes must be read once, so the stream time is the floor):
  host: pack W_h.T / hidden.T / masks / W_alpha into ONE setup blob,
        pre-rearranged so it is a single contiguous DMA. Setup DMAs are
        issued before the stream DMAs so their completion-semaphore lanes
        clear first (a late setup DMA sharing a lane with stream DMAs
        serializes the whole pipeline).
  setup (no DRAM round-trips): w_h = hidden @ W_h.T on PE, then per-batch
        partition-broadcast of w_h rows via one-hot PE matmuls.
  per batch b (software-pipelined, p_att phase leads att_feats phase):
    p_att [128,16,512] in ONE 4MB DMA (32KB/partition contiguous):
      DVE add (w_h bcast, 2 halves) -> ACT tanh (bf16) -> DVE
      scalar_tensor_tensor vs W_alpha (accum) -> scores[128,16]
    scores: + mask, ACT exp (accum rowsum, f32r out), PE total-sum,
      DVE reciprocal.  exp/sum are issued AFTER af_phase(b-1) so the sum
      matmul never sits ahead of batch b-1's att matmuls in the PE FIFO.
    att_feats [128,8,1024] f32r in TWO 4MB DMAs: PE matmuls (expt col
      stationary) accumulating att[1,1024] in PSUM -> DVE scale by
      1/sum -> out row (SWDGE queue, off the stream ring).
  Batch 7's att_feats stream is split into 1MB DMAs so the final matmuls
  pipeline with the last data and the kernel tail stays short.
"""

import numpy as np

B, N, H, A = 64, 2048, 1024, 512
NCORES = 8
BLOC = B // NCORES  # batches per core

P = 128
NT = N // P            # 16 n-columns per partition
HC = H // P            # 8 h-chunks

# setup blob column offsets (f32 columns)
BL_WHT = 0                 # [128, 8*512] W_h.T chunks
BL_HID = BL_WHT + HC * A   # [128, 64]   hidden.T chunks
BL_MSK = BL_HID + HC * BLOC  # [128, 128] masks
BL_WA = BL_MSK + BLOC * NT   # [128, 512] W_alpha broadcast
BL_W = BL_WA + A

_NC_CACHE = {}


def _free_bcast(bass_mod, ap, repeat):
    """[P, F] AP -> [P, repeat, F] AP with 0-stride middle dim."""
    return bass_mod.AP(
        tensor=ap.tensor,
        offset=ap.offset,
        ap=[ap.ap[0], [0, repeat], *ap.ap[1:]],
    )


def _build_nc():
    import concourse.bass as bass
    import concourse.mybir as mybir
    import concourse.tile as tile
    from concourse import bacc

    dt = mybir.dt
    f32, f32r, bf16 = dt.float32, dt.float32r, dt.bfloat16
    AF = mybir.ActivationFunctionType
    OP = mybir.AluOpType

    nc = bacc.Bacc("TRN2", target_bir_lowering=False, debug=False,
                   num_devices=NCORES)

    af = nc.dram_tensor("att_feats", [BLOC, N, H], f32r, kind="ExternalInput").ap()
    pa = nc.dram_tensor("p_att_feats", [BLOC, N, A], f32, kind="ExternalInput").ap()
    blob = nc.dram_tensor("setup_blob", [P, BL_W], f32, kind="ExternalInput").ap()
    # onehot[k, b*128+p] = 1 if k == b else 0 (for the w_h row broadcast)
    oh = nc.dram_tensor("onehot", [BLOC, BLOC * P], f32, kind="ExternalInput").ap()
    out = nc.dram_tensor("att_out", [BLOC, H], f32, kind="ExternalOutput").ap()

    with tile.TileContext(nc) as tc:
        with (
            tc.tile_pool(name="consts", bufs=1) as consts,
            tc.tile_pool(name="patt", bufs=2) as patt_pool,
            tc.tile_pool(name="alpha", bufs=2) as alpha_pool,
            tc.tile_pool(name="afp", bufs=2) as af_pool,
            tc.tile_pool(name="small", bufs=3) as small,
            tc.tile_pool(name="arow", bufs=2) as arow,
            tc.tile_pool(name="psmisc", bufs=2, space="PSUM") as psmisc,
            tc.tile_pool(name="psatt", bufs=6, space="PSUM") as psatt,
        ):
            pa_r = [pa[b, :, :].rearrange("(p c) a -> p c a", c=NT)
                    for b in range(BLOC)]
            af_r = [af[b, :, :].rearrange("(p c) h -> p c h", c=NT)
                    for b in range(BLOC)]

            # patt(0) stream DMA first so the stream ring starts instantly.
            pt0 = patt_pool.tile([P, NT, A], f32, tag="patt", name="patt0")
            nc.sync.dma_start(out=pt0, in_=pa_r[0][:, :, :])

            # ---------------- setup ----------------
            bl = consts.tile([P, BL_W], f32)
            nc.sync.dma_start(out=bl, in_=blob[:, :])
            onehot = consts.tile([BLOC, BLOC * P], f32)
            nc.sync.dma_start(out=onehot, in_=oh[:, :])

            ones_col = consts.tile([P, 1], f32)
            nc.vector.memset(ones_col, 1.0)
            wa_bf = consts.tile([P, A], bf16)
            nc.vector.tensor_copy(wa_bf, bl[:, BL_WA:BL_WA + A])

            # w_h = hidden @ W_h.T : [8, 512]
            wh_ps = psmisc.tile([BLOC, A], f32, tag="mm")
            for hc in range(HC):
                nc.tensor.matmul(
                    wh_ps,
                    lhsT=bl[:, BL_HID + hc * BLOC:BL_HID + (hc + 1) * BLOC],
                    rhs=bl[:, BL_WHT + hc * A:BL_WHT + (hc + 1) * A],
                    start=(hc == 0), stop=(hc == HC - 1))
            whall_sb = consts.tile([BLOC, A], f32)
            nc.scalar.activation(whall_sb, wh_ps, AF.Copy)

            # per-batch w_h row broadcast to [128, 512] via one-hot matmuls
            whb = []
            for b in range(BLOC):
                bps = psmisc.tile([P, A], f32, tag="mm", name=f"whbps{b}")
                nc.tensor.matmul(bps, lhsT=onehot[:, b * P:(b + 1) * P],
                                 rhs=whall_sb, start=True, stop=True)
                t = consts.tile([P, A], f32, name=f"whb{b}", tag=f"whb{b}")
                nc.scalar.activation(t, bps, AF.Copy)
                whb.append(t)

            # ---------------- main loop (software-pipelined) ----------------
            def patt_front(b):
                """DMA + add + tanh + stt + mask -> scores tile."""
                if b == 0:
                    pt = pt0
                else:
                    pt = patt_pool.tile([P, NT, A], f32, tag="patt",
                                        name=f"patt{b}")
                    nc.sync.dma_start(out=pt, in_=pa_r[b][:, :, :])

                scores = small.tile([P, NT], f32, tag="scores",
                                    name=f"scores{b}")
                HALF = NT // 2
                whb_b = _free_bcast(bass, whb[b][:, :], HALF)
                for st in range(2):
                    sl = pt[:, st * HALF:(st + 1) * HALF, :]
                    nc.vector.tensor_tensor(out=sl, in0=sl, in1=whb_b,
                                            op=OP.add)
                    ab = alpha_pool.tile([P, HALF, A], bf16, tag="alpha",
                                         name=f"alpha{b}_{st}")
                    nc.scalar.activation(ab, sl, AF.Tanh)
                    for c in range(HALF):
                        col = st * HALF + c
                        # out = (ab * 1) * wa ; accum_out = row-sum -> scores
                        nc.vector.scalar_tensor_tensor(
                            out=ab[:, c, :], in0=ab[:, c, :], scalar=1.0,
                            in1=wa_bf, op0=OP.mult, op1=OP.mult,
                            accum_out=scores[:, col:col + 1],
                        )

                nc.vector.tensor_tensor(
                    out=scores, in0=scores,
                    in1=bl[:, BL_MSK + b * NT:BL_MSK + (b + 1) * NT],
                    op=OP.add)
                return scores

            def patt_fin(b, scores):
                """exp + total-sum + reciprocal. Issued AFTER af_phase(b-1)
                so the sum matmul never sits ahead of batch b-1's att
                matmuls in the PE FIFO."""
                expt = small.tile([P, NT], f32r, tag="expt", name=f"expt{b}")
                rowsum = small.tile([P, 1], f32, tag="rowsum", name=f"rowsum{b}")
                nc.scalar.activation(expt, scores, AF.Exp, accum_out=rowsum)

                sum_ps = psmisc.tile([1, 1], f32, tag="mm", name=f"sum_ps{b}")
                nc.tensor.matmul(sum_ps, lhsT=rowsum, rhs=ones_col,
                                 start=True, stop=True)
                inv = small.tile([1, 1], f32, tag="inv", name=f"inv{b}")
                nc.vector.reciprocal(inv, sum_ps)
                return expt, inv

            def af_phase(b, expt, inv):
                att_lo = psatt.tile([1, A], f32, tag="att", name=f"attlo{b}")
                att_hi = psatt.tile([1, A], f32, tag="att", name=f"atthi{b}")
                # last batch: small supertiles so the final matmuls pipeline
                # with the last-arriving data (short kernel tail)
                sup = 8 if b < BLOC - 1 else 2
                for st2 in range(NT // sup):
                    aft = af_pool.tile([P, sup, H], f32r, tag="af",
                                       name=f"af{b}_{st2}")
                    nc.sync.dma_start(
                        out=aft,
                        in_=af_r[b][:, st2 * sup:(st2 + 1) * sup, :],
                    )
                    for c in range(sup):
                        t = st2 * sup + c
                        lhs = expt[:, t:t + 1]
                        nc.tensor.matmul(att_lo, lhsT=lhs,
                                         rhs=aft[:, c, 0:A],
                                         start=(t == 0), stop=(t == NT - 1))
                        nc.tensor.matmul(att_hi, lhsT=lhs,
                                         rhs=aft[:, c, A:H],
                                         start=(t == 0), stop=(t == NT - 1))

                att_row = arow.tile([1, H], f32, tag="attrow",
                                    name=f"attrow{b}")
                nc.vector.tensor_scalar_mul(att_row[:, 0:A], att_lo, inv)
                nc.vector.tensor_scalar_mul(att_row[:, A:H], att_hi, inv)
                # SWDGE queue (idle GpSimd): keeps the tiny output store off
                # the stream ring so it can't head-of-line-block anything.
                nc.gpsimd.dma_start(out=out[b:b + 1, :], in_=att_row)

            fin = {}
            for b in range(BLOC):
                scores_b = patt_front(b)
                if b >= 1:
                    af_phase(b - 1, *fin.pop(b - 1))
                fin[b] = patt_fin(b, scores_b)
            af_phase(BLOC - 1, *fin.pop(BLOC - 1))

    nc.compile()
    return nc


def _get_nc():
    if "nc" not in _NC_CACHE:
        _NC_CACHE["nc"] = _build_nc()
    return _NC_CACHE["nc"]


def kernel(hidden_states, att_feats, p_att_feats, att_masks, W_h, W_alpha):
    from concourse.bass_utils import run_bass_kernel_spmd

    nc = _get_nc()
    hidden_states = np.ascontiguousarray(hidden_states, dtype=np.float32)
    att_feats = np.ascontiguousarray(att_feats, dtype=np.float32)
    p_att_feats = np.ascontiguousarray(p_att_feats, dtype=np.float32)
    att_masks = np.ascontiguousarray(att_masks, dtype=np.float32)
    W_h = np.ascontiguousarray(W_h, dtype=np.float32)
    W_alpha = np.asarray(W_alpha, dtype=np.float32).reshape(1, A)

    # whT_r[p, hc, a] = W_h[a, hc*128+p]
    whT_r = W_h.T.reshape(HC, P, A).transpose(1, 0, 2).reshape(P, HC * A)
    wa_b = np.broadcast_to(W_alpha, (P, A))
    onehot_host = np.zeros((BLOC, BLOC * P), dtype=np.float32)
    for b in range(BLOC):
        onehot_host[b, b * P:(b + 1) * P] = 1.0

    in_maps = []
    for i in range(NCORES):
        s = slice(i * BLOC, (i + 1) * BLOC)
        hs = hidden_states[s]  # [BLOC, H]
        # hidT_r[p, hc*8+b] = hidden[b, hc*128+p]
        hidT_r = hs.T.reshape(HC, P, BLOC).transpose(1, 0, 2).reshape(P, HC * BLOC)
        masks_r = att_masks[s].reshape(BLOC, P, NT).transpose(1, 0, 2)
        blob = np.concatenate(
            [whT_r, hidT_r, masks_r.reshape(P, BLOC * NT), wa_b], axis=1)
        in_maps.append({
            "att_feats": att_feats[s],
            "p_att_feats": p_att_feats[s],
            "setup_blob": np.ascontiguousarray(blob),
            "onehot": onehot_host,
        })

    global _LAST_IN_MAPS
    _LAST_IN_MAPS = in_maps
    res = run_bass_kernel_spmd(nc, in_maps, core_ids=list(range(NCORES)))
    return np.concatenate(
        [res.results[i]["att_out"] for i in range(NCORES)], axis=0
    ).astype(np.float32)


_LAST_IN_MAPS = None
